# revision 1
# baseline (speedup 1.0000x reference)
"""TRN2 Bass kernel for nn_CrossAttention_71287867179098.

Cross attention: out = softmax((x1@Wq) @ (x2@Wk)^T / sqrt(d)) @ (x2@Wv)
Shapes: x_1 [4096,1024], x_2 [4096,1024], W_* [1024,1024], out [4096,1024], fp32.

Sharding: query rows (x_1) split across 8 cores (512 rows each); x_2 and
weights replicated. Each core runs one-pass flash attention over kv chunks.

Algebra: kv-side projections are folded out so the 4096-long kv axis is hit
by exactly one matmul per side of the softmax:
  scores = G @ x2^T where G = x1 @ Wq @ Wk^T
  out    = ((P @ x2) @ Wv) / sums

Precision: all pre-softmax matmuls run in fp32r (fp32 storage, RNE to 11
explicit mantissa bits, full fp16-rate on the PE).  Single-pass fp32r is
viable because the huge rank-1 structure of the scores (uniform-positive
weights => G entries up to ~28000) is removed exactly and added back at
fp32 precision:
  Wq = 0.5 + dq, Wk = 0.5 + dk  (centered weights, |dq|<=0.5)
  Qt = x1 @ dq ; Gt = Qt @ dk^T                       (fp32r chain, small)
  G2 = Gt + 0.5*A2 (x) dkbar     A2 = rowsum(x1), dkbar = rowsum(Wk)-512
  scores = G2 @ x2^T + A1 (x) B1  A1 = x1@(0.5*rowsum(Wq)), B1 = rowsum(x2)
The A1/B1 rank-1 term is added into the score PSUM as one tiny extra
matmul (contraction 3: A1h,A1h,A1l x B1h,B1l,B1h in fp32r hi/lo), with
A1/B1 computed from unrounded fp32 data by 2-stage reductions.
"""

import sys

sys.path.insert(0, "/opt/trn_rl_repo")

import numpy as np

import concourse.bass as bass
from concourse import bacc
import concourse.mybir as mybir
import concourse.tile as tile
from concourse.bass_utils import run_bass_kernel_spmd
from concourse.masks import make_identity

F32 = mybir.dt.float32
F32R = mybir.dt.float32r
F16 = mybir.dt.float16
AX = mybir.AxisListType
ALU = mybir.AluOpType
ACTF = mybir.ActivationFunctionType

P = 128
D = 1024          # d_in == d_kq == d_v
CO = D // P       # contraction chunks (8)
NQ = 512          # query rows per core
QT = NQ // P      # query tiles per core (4)
NKV = 4096
CHUNK = 512       # kv rows per chunk
NCH = NKV // CHUNK
JO = CHUNK // P   # kv subtiles per chunk (4)
NCORES = 8
INV_SQRT_D = 1.0 / 32.0


def build_kernel() -> bass.Bass:
    nc = bacc.Bacc(target_bir_lowering=False)
    x1_d = nc.dram_tensor("x1s", [NQ, D], F32, kind="ExternalInput")
    x2_d = nc.dram_tensor("x2", [NKV, D], F32, kind="ExternalInput")
    wq_d = nc.dram_tensor("Wq", [D, D], F32, kind="ExternalInput")
    wk_d = nc.dram_tensor("Wk", [D, D], F32, kind="ExternalInput")
    wv_d = nc.dram_tensor("Wv", [D, D], F32, kind="ExternalInput")
    out_d = nc.dram_tensor("out", [NQ, D], F32, kind="ExternalOutput")
    scr_u = nc.dram_tensor("scr_u", [D], F32)            # u1 row bounce
    scr_a = nc.dram_tensor("scr_a", [2, NQ], F32)        # A1/A2 row bounce
    scr_b = nc.dram_tensor("scr_b", [NCH, CHUNK], F32)   # B1 row bounce
    scr_b2 = nc.dram_tensor("scr_b2", [NCH, 2, CHUNK], F32)  # B1 h/l rows
    scr_a2 = nc.dram_tensor("scr_a2", [2, NQ], F32)      # A1 h/l rows

    with tile.TileContext(nc) as tc:
        with (
            tc.tile_pool(name="const", bufs=1) as constp,
            tc.tile_pool(name="persist", bufs=1) as persist,
            tc.tile_pool(name="stats", bufs=8) as stats,
            tc.tile_pool(name="psT", bufs=2, space="PSUM") as psT,
        ):
            ident32 = constp.tile([P, P], F32)
            make_identity(nc, ident32[:])
            ident16 = constp.tile([P, P], F16)
            make_identity(nc, ident16[:])
            identr = constp.tile([P, P], F32R)
            nc.scalar.activation(identr[:], ident32[:], ACTF.Copy)

            # ---- persistent operands ----
            g2r = persist.tile([P, CO, NQ], F32R)    # G2^T [e, i]
            at3 = persist.tile([3, NQ], F32R)        # [A1h; A1h; A1l]
            u1rep = persist.tile([P, D], F32)        # u1 bcast along i-part
            a2rep = persist.tile([P, NQ], F32)       # A2 bcast along e-part
            dkb = persist.tile([P, CO], F32)         # 0.5*(kbar-512) per e
            t_acc = [persist.tile([P, D], F32, name=f"t_acc{q}") for q in range(QT)]
            m_cur = [stats.tile([P, 1], F32, tag="m", name=f"m0_{q}") for q in range(QT)]
            s_cur = [stats.tile([P, 1], F32, tag="s", name=f"s0_{q}") for q in range(QT)]
            for q in range(QT):
                nc.gpsimd.memset(t_acc[q][:], 0.0)
                nc.gpsimd.memset(m_cur[q][:], -1e30)
                nc.gpsimd.memset(s_cur[q][:], 0.0)

            # ---------- phase 0 ----------
            ones32 = constp.tile([P, 1], F32)
            nc.vector.memset(ones32[:], 1.0)
            with (
                tc.tile_pool(name="qtrp", bufs=1) as qtrp,
                tc.tile_pool(name="ph0c", bufs=1) as ph0c,
            ):
                qtr = qtrp.tile([P, CO, NQ], F32R)   # Qt^T [d, i]
                a12 = ph0c.tile([P, QT, 2], F32)
                u1h = ph0c.tile([P, CO], F32)
                x1t32 = ph0c.tile([P, CO, NQ], F32)  # raw x1^T for the A1 mm
                with (
                    tc.tile_pool(name="ph0a", bufs=1) as ph0a,
                    tc.tile_pool(name="psMM", bufs=2, space="PSUM") as psMM,
                ):
                  with tc.tile_pool(name="x1p", bufs=1) as x1p:

                      x1c = x1p.tile([P, QT, D], F32)
                      nc.sync.dma_start(
                          x1c[:, 0:2, :],
                          x1_d[0:2 * P, :].rearrange("(io p) c -> p io c", p=P),
                      )
                      nc.sync.dma_start(
                          x1c[:, 2:4, :],
                          x1_d[2 * P:, :].rearrange("(io p) c -> p io c", p=P),
                      )
                      wqf = x1p.tile([P, CO, D], F32)
                      nc.sync.dma_start(
                          wqf[:], wq_d.rearrange("(co p) d -> p co d", p=P)
                      )

                      # centered fp32r Wq (Qt critical path: first DVE op)
                      dqr = ph0a.tile([P, CO, D], F32R)
                      nc.vector.tensor_scalar(
                          dqr[:], wqf[:], -0.5, None, ALU.add
                      )

                      # A2 = rowsum(x1) 2-stage
                      a2a = x1p.tile([P, QT, CO], F32)
                      nc.vector.tensor_reduce(
                          a2a[:], x1c[:].rearrange("p io (a b) -> p io a b", b=P),
                          AX.X, ALU.add,
                      )
                      nc.vector.tensor_reduce(a12[:, :, 1:2], a2a[:], AX.X, ALU.add)

                      # qbar 2-stage -> u1 = 0.5*qbar  [c-part, co]
                      qba = x1p.tile([P, CO, CO], F32)
                      nc.vector.tensor_reduce(
                          qba[:], wqf[:].rearrange("p co (a b) -> p co a b", b=P),
                          AX.X, ALU.add,
                      )
                      nc.vector.tensor_reduce(u1h[:].unsqueeze(2), qba[:], AX.X, ALU.add)
                      nc.vector.tensor_scalar_mul(u1h[:], u1h[:], 0.5)

                      # x1^T (first PE work, io-major so half-DMA overlaps):
                      # fp32r copy for Qt, raw fp32 for A1
                      x1t = ph0a.tile([P, CO, NQ], F32R)
                      for io in range(QT):
                          for ch in range(2):
                              pst = psT.tile([P, NQ], F32, tag="pst", bufs=2,
                                             name=f"px1_{io}_{ch}")
                              for ci in range(4):
                                  co = ch * 4 + ci
                                  nc.tensor.transpose(
                                      pst[:, ci * P:(ci + 1) * P],
                                      x1c[:, io, co * P:(co + 1) * P], ident32[:],
                                  )
                              pv = pst[:].rearrange("p (a b) -> p a b", b=P)
                              nc.scalar.activation(
                                  x1t[:, ch * 4:(ch + 1) * 4,
                                      io * P:(io + 1) * P], pv, ACTF.Copy,
                              )
                              nc.vector.tensor_copy(
                                  x1t32[:, ch * 4:(ch + 1) * 4,
                                        io * P:(io + 1) * P], pv,
                              )

                      # A2 row -> a2rep
                      psa2 = psMM.tile([1, NQ], F32, tag="psa2", bufs=1)
                      for io in range(QT):
                          nc.tensor.transpose(
                              psa2[:, io * P:(io + 1) * P], a12[:, io, 1:2],
                              ident32[:],
                          )
                      a2row = ph0c.tile([1, NQ], F32)
                      nc.vector.tensor_copy(a2row[:], psa2[:])
                      nc.gpsimd.partition_broadcast(a2rep[:], a2row[:])

                      # Qt^T [d, i] = dq-lhsT @ x1t
                      for dc in range(CO):
                          ps = psMM.tile([P, NQ], F32, tag="ps", name=f"psq_{dc}")
                          for cc in range(CO):
                              nc.tensor.matmul(
                                  ps[:],
                                  dqr[:, cc, dc * P:(dc + 1) * P],
                                  x1t[:, cc, :],
                                  start=(cc == 0),
                                  stop=(cc == CO - 1),
                              )
                          nc.scalar.activation(qtr[:, dc, :], ps[:], ACTF.Copy)

                  with tc.tile_pool(name="ph0b", bufs=1) as ph0b:
                      wkf = ph0b.tile([P, CO, D], F32)
                      nc.sync.dma_start(
                          wkf[:], wk_d.rearrange("(co p) d -> p co d", p=P)
                      )

                      # kbar 2-stage -> dkb = 0.5*kbar - 256
                      kba = ph0b.tile([P, CO, CO], F32)
                      nc.vector.tensor_reduce(
                          kba[:],
                          wkf[:].rearrange("p co (a b) -> p co a b", b=P),
                          AX.X, ALU.add,
                      )
                      nc.vector.tensor_reduce(
                          dkb[:].unsqueeze(2), kba[:], AX.X, ALU.add
                      )
                      nc.vector.tensor_scalar(
                          dkb[:], dkb[:], 0.5, -256.0, ALU.mult, ALU.add
                      )
                      # dk^T fp32r [d, e]: transpose + center in psum copy
                      # (half-major so G2's ec 0-3 can start after 8 copies)
                      dkt = ph0b.tile([P, CO, D], F32R)
                      for half in range(2):
                          for dc in range(CO):
                              pst = psT.tile(
                                  [P, NQ], F32, tag="pst", bufs=2,
                                  name=f"pwk_{dc}_{half}",
                              )
                              for ci in range(4):
                                  co = half * 4 + ci
                                  nc.tensor.transpose(
                                      pst[:, ci * P:(ci + 1) * P],
                                      wkf[:, co, dc * P:(dc + 1) * P],
                                      ident32[:],
                                  )
                              if dc % 2 == 0:
                                  nc.scalar.activation(
                                      dkt[:, dc, half * NQ:(half + 1) * NQ],
                                      pst[:], ACTF.Copy, bias=-0.5,
                                  )
                              else:
                                  nc.vector.tensor_scalar(
                                      dkt[:, dc, half * NQ:(half + 1) * NQ],
                                      pst[:], -0.5, None, ALU.add,
                                  )

                      # A1 = x1 @ u1 as exact-fp32 PE matmul -> [1, 512] row
                      psA1 = psMM.tile([1, NQ], F32, tag="psA1", bufs=1)
                      for cc in range(CO):
                          nc.tensor.matmul(
                              psA1[:], u1h[:, cc:cc + 1], x1t32[:, cc, :],
                              start=(cc == 0), stop=(cc == CO - 1),
                          )
                      a1row = ph0c.tile([1, NQ], F32)
                      nc.vector.tensor_copy(a1row[:], psA1[:])
                      a1h = ph0c.tile([1, NQ], F32R)
                      nc.vector.tensor_copy(a1h[:], a1row[:])
                      a1l = ph0c.tile([1, NQ], F32R)
                      nc.vector.scalar_tensor_tensor(
                          a1l[:], a1row[:], 1.0, a1h[:], ALU.mult, ALU.subtract
                      )
                      nc.sync.dma_start(scr_a2[0:1, :], a1h[:].bitcast(F32))
                      nc.sync.dma_start(scr_a2[1:2, :], a1l[:].bitcast(F32))
                      # at3 rows [A1h, A1l, A1h]
                      atsrc = ph0c.tile([3, NQ], F32)
                      nc.sync.dma_start(atsrc[0:1, :], scr_a2[0:1, :])
                      nc.sync.dma_start(atsrc[1:2, :], scr_a2[1:2, :])
                      nc.sync.dma_start(atsrc[2:3, :], scr_a2[0:1, :])
                      nc.vector.tensor_copy(at3[:], atsrc[:])

                      # G2^T [e, i] = dk-lhsT @ qtr  (+ 0.5*dkbar (x) A2)
                      for ec in range(CO):
                          ps = psMM.tile([P, NQ], F32, tag="ps", name=f"psg_{ec}")
                          for dc in range(CO):
                              nc.tensor.matmul(
                                  ps[:],
                                  dkt[:, dc, ec * P:(ec + 1) * P],
                                  qtr[:, dc, :],
                                  start=(dc == 0),
                                  stop=(dc == CO - 1),
                              )
                          nc.vector.scalar_tensor_tensor(
                              g2r[:, ec, :], a2rep[:], dkb[:, ec:ec + 1],
                              ps[:], ALU.mult, ALU.add,
                          )

            # ---------- phase 1: flash attention over kv chunks ----------
            with tc.tile_pool(name="wvhp", bufs=1) as wvhp:
              wv_h = wvhp.tile([P, CO, D], F16)
              with (
                tc.tile_pool(name="wvp", bufs=1) as wvp,
                  tc.tile_pool(name="x2cp", bufs=2) as x2cp,
                  tc.tile_pool(name="x2np", bufs=2) as x2np,
                  tc.tile_pool(name="x2tp", bufs=2) as x2tp,
                  tc.tile_pool(name="btp", bufs=2) as btp,
                  tc.tile_pool(name="pp", bufs=2) as ppool,
                  tc.tile_pool(name="psS", bufs=2, space="PSUM") as psS,
                  tc.tile_pool(name="psO", bufs=2, space="PSUM") as psO,
                  tc.tile_pool(name="psTP", bufs=1, space="PSUM") as psTP,
              ):

                  def prepare_dma(t):
                      """DMA chunk t."""
                      x2c = x2cp.tile([P, JO, D], F32, tag="x2c", name=f"x2c_{t}")
                      src = x2_d[t * CHUNK:(t + 1) * CHUNK, :]
                      nc.sync.dma_start(
                          x2c[:], src.rearrange("(jo p) c -> p jo c", p=P)
                      )
                      b1a = x2cp.tile([P, JO, CO], F32, tag="b1a", name=f"b1a_{t}")
                      b1c = x2cp.tile([P, JO, 1], F32, tag="b1c", name=f"b1c_{t}")
                      return x2c, b1a, b1c

                  def jo_red(x2c, b1a, jo):
                      """One jo-slice of the B1 stage-1 reduce (keeps DVE
                      queue latency low between softmax slots)."""
                      nc.vector.tensor_reduce(
                          b1a[:, jo:jo + 1, :],
                          x2c[:, jo:jo + 1, :]
                          .rearrange("p jo (a b) -> p jo a b", b=P),
                          AX.X, ALU.add,
                      )

                  def prepare_round(t, x2c):
                      """fp16 cast of the natural-layout chunk (T-matmul rhs)."""
                      x2n = x2np.tile([P, JO, D], F16, tag="x2n", name=f"x2n_{t}")
                      nc.scalar.activation(x2n[:], x2c[:], ACTF.Copy)
                      return x2n

                  def prepare_trans(t, x2c, part, all_scalar=False):
                      """Two co-banks of x2^T transposes + rounded copies."""
                      if part == 0:
                          prepare_trans.cur = x2tp.tile(
                              [P, CO, CHUNK], F32R, tag="x2t", name=f"x2t_{t}"
                          )
                      x2t = prepare_trans.cur
                      for co in (2 * part, 2 * part + 1):
                          pst = psT.tile([P, CHUNK], F32, tag="pst", bufs=2,
                                         name=f"pst_{t}_{co}")
                          for jo in range(JO):
                              nc.tensor.transpose(
                                  pst[:, jo * P:(jo + 1) * P],
                                  x2c[:, jo, co * P:(co + 1) * P], ident32[:],
                              )
                          if all_scalar or co % 2 == 0:
                              nc.scalar.activation(x2t[:, co, :], pst[:], ACTF.Copy)
                          else:
                              nc.vector.tensor_copy(x2t[:, co, :], pst[:])
                      return x2t

                  def b_path(t, b1a, b1c):
                      """B1 [j-part] -> bt3 [3, 512] rows via dram bounce."""
                      nc.vector.tensor_reduce(b1c[:], b1a[:], AX.X, ALU.add)
                      psb = psT.tile([JO, P], F32, tag="psb", bufs=1,
                                     name=f"psb_{t}")
                      nc.tensor.transpose(psb[:], b1c[:, :, 0], ident32[:])
                      b4 = btp.tile([JO, P], F32, tag="b4", name=f"b4_{t}")
                      nc.vector.tensor_copy(b4[:], psb[:])
                      b4h = btp.tile([JO, P], F32R, tag="b4h", name=f"b4h_{t}")
                      nc.vector.tensor_copy(b4h[:], b4[:])
                      b4l = btp.tile([JO, P], F32R, tag="b4l", name=f"b4l_{t}")
                      nc.vector.scalar_tensor_tensor(
                          b4l[:], b4[:], 1.0, b4h[:], ALU.mult, ALU.subtract
                      )
                      nc.sync.dma_start(
                          scr_b2[t, 0].rearrange("(a b) -> a b", a=JO),
                          b4h[:].bitcast(F32),
                      )
                      nc.sync.dma_start(
                          scr_b2[t, 1].rearrange("(a b) -> a b", a=JO),
                          b4l[:].bitcast(F32),
                      )
                      # bt3 rows [B1h, B1h, B1l] (pairs at3 [A1h, A1l, A1h])
                      btsrc = btp.tile([3, CHUNK], F32, tag="btsrc",
                                       name=f"btsrc_{t}")
                      nc.sync.dma_start(btsrc[0:1, :], scr_b2[t, 0:1, :])
                      nc.sync.dma_start(btsrc[1:2, :], scr_b2[t, 0:1, :])
                      nc.sync.dma_start(btsrc[2:3, :], scr_b2[t, 1:2, :])
                      bt3 = btp.tile([3, CHUNK], F32R, tag="bt3", name=f"bt3_{t}")
                      nc.vector.tensor_copy(bt3[:], btsrc[:])
                      return bt3

                  def scores(t, q, x2t, bt3):
                      ps_s = psS.tile([P, CHUNK], F32, tag="ps_s",
                                      name=f"ps_s_{t}_{q}")
                      for cc in range(CO):
                          nc.tensor.matmul(
                              ps_s[:],
                              g2r[:, cc, q * P:(q + 1) * P],
                              x2t[:, cc, :],
                              start=(cc == 0),
                              stop=False,
                          )
                      nc.tensor.matmul(
                          ps_s[:], at3[:, q * P:(q + 1) * P], bt3[:],
                          start=False, stop=True,
                      )
                      return ps_s

                  def softmax_t(t, q, ps_s, x2n):
                      rm = stats.tile([P, 1], F32, tag="rm")
                      nc.vector.reduce_max(rm[:], ps_s[:], axis=AX.X)
                      m_new = stats.tile([P, 1], F32, tag="m")
                      nc.vector.tensor_tensor(
                          m_new[:], m_cur[q][:], rm[:], ALU.max
                      )
                      bias = stats.tile([P, 1], F32, tag="bias")
                      nc.vector.tensor_scalar_mul(bias[:], m_new[:], -INV_SQRT_D)
                      fsc = stats.tile([P, 1], F32, tag="fsc")
                      nc.scalar.activation(
                          fsc[:], m_cur[q][:], ACTF.Exp,
                          bias=bias[:], scale=INV_SQRT_D,
                      )
                      p_c = ppool.tile([P, CHUNK], F16, tag="p_c",
                                       name=f"p_c_{t}_{q}")
                      rs = stats.tile([P, 1], F32, tag="rs")
                      nc.scalar.activation(
                          p_c[:], ps_s[:], ACTF.Exp,
                          bias=bias[:], scale=INV_SQRT_D, accum_out=rs[:],
                      )
                      s_new = stats.tile([P, 1], F32, tag="s")
                      nc.vector.scalar_tensor_tensor(
                          s_new[:], s_cur[q][:], fsc[:], rs[:], ALU.mult, ALU.add
                      )
                      m_cur[q] = m_new
                      s_cur[q] = s_new

                      # P^T tiles then copy out of psum
                      pstp = psTP.tile([P, CHUNK], F16, tag="pstp")
                      for jt in range(JO):
                          nc.tensor.transpose(
                              pstp[:, jt * P:(jt + 1) * P],
                              p_c[:, jt * P:(jt + 1) * P], ident16[:],
                          )
                      p_t = ppool.tile([P, CHUNK], F16, tag="p_t",
                                       name=f"p_t_{t}_{q}")
                      nc.vector.tensor_copy(p_t[:], pstp[:])
                      for dh in range(2):
                          ps_o = psO.tile([P, 512], F32, tag="ps_o")
                          for jt in range(JO):
                              nc.tensor.matmul(
                                  ps_o[:],
                                  p_t[:, jt * P:(jt + 1) * P],
                                  x2n[:, jt, dh * 512:(dh + 1) * 512],
                                  start=(jt == 0),
                                  stop=(jt == JO - 1),
                              )
                          dst = t_acc[q][:, dh * 512:(dh + 1) * 512]
                          nc.vector.scalar_tensor_tensor(
                              dst, dst, fsc[:], ps_o[:], ALU.mult, ALU.add
                          )

                  def ph2_q(q, tt, outp2, pspT, pspM):
                      """Per-q output path: tn, T^T tiles, O = tt @ Wv, store."""
                      rcp = stats.tile([P, 1], F32, tag="rcp")
                      nc.vector.reciprocal(rcp[:], s_cur[q][:])
                      tn = outp2.tile([P, D], F16, tag="tn", name=f"tn_{q}")
                      if q % 2 == 0:
                          nc.scalar.activation(
                              tn[:], t_acc[q][:], ACTF.Copy, scale=rcp[:]
                          )
                      else:
                          nc.vector.tensor_scalar(
                              tn[:], t_acc[q][:], rcp[:], None, ALU.mult
                          )
                      for half in range(2):
                          pstp2 = pspT.tile(
                              [P, CHUNK], F16, tag="pstp2",
                              name=f"pstp2_{q}_{half}",
                          )
                          for ci in range(4):
                              cc = half * 4 + ci
                              nc.tensor.transpose(
                                  pstp2[:, ci * P:(ci + 1) * P],
                                  tn[:, cc * P:(cc + 1) * P], ident16[:],
                              )
                          if half == 0:
                              nc.vector.tensor_copy(
                                  tt[:, 0:4, q * P:(q + 1) * P],
                                  pstp2[:].rearrange("p (a b) -> p a b", b=P),
                              )
                          else:
                              nc.scalar.activation(
                                  tt[:, 4:8, q * P:(q + 1) * P],
                                  pstp2[:].rearrange("p (a b) -> p a b", b=P),
                                  ACTF.Copy,
                              )
                      out_ap = out_d.rearrange("(qo p) d -> p qo d", p=P)
                      o_sb = outp2.tile([P, D], F32, tag="osb", name=f"osb_{q}")
                      for dh in range(2):
                          ps = pspM.tile([P, 512], F32, tag="pso2",
                                        name=f"pso2_{q}_{dh}")
                          for cc in range(CO):
                              nc.tensor.matmul(
                                  ps[:],
                                  tt[:, cc, q * P:(q + 1) * P],
                                  wv_h[:, cc, dh * 512:(dh + 1) * 512],
                                  start=(cc == 0),
                                  stop=(cc == CO - 1),
                              )
                          if dh == 0:
                              nc.vector.tensor_copy(o_sb[:, 0:512], ps[:])
                          else:
                              nc.scalar.activation(
                                  o_sb[:, 512:1024], ps[:], ACTF.Copy
                              )
                          nc.sync.dma_start(
                              out_ap[:, q, dh * 512:(dh + 1) * 512],
                              o_sb[:, dh * 512:(dh + 1) * 512],
                          )

                  # stage chunk 0 before the loop
                  d0 = prepare_dma(0)
                  x2n0 = prepare_round(0, d0[0])
                  for jo in range(JO):
                      jo_red(d0[0], d0[1], jo)
                  for part in range(4):
                      x2t0 = prepare_trans(0, d0[0], part)
                  bt30 = b_path(0, d0[1], d0[2])
                  dmas = [d0]
                  chunks = [(x2n0, x2t0, bt30)]

                  # Wv load + fp16 cast in halves (off critical path)
                  for half in range(2):
                      wvf = wvp.tile([P, 4, D], F32, tag="wvf", name=f"wvf_{half}")
                      nc.sync.dma_start(
                          wvf[:],
                          wv_d.rearrange("(co p) d -> p co d", p=P)[
                              :, half * 4:(half + 1) * 4, :
                          ],
                      )
                      nc.scalar.activation(
                          wv_h[:, half * 4:(half + 1) * 4, :], wvf[:], ACTF.Copy
                      )

                  tt2 = None
                  carry = None
                  for t in range(NCH):
                      x2n, x2t, bt3 = chunks[t]
                      more = t + 1 < NCH
                      ps0 = scores(t, 0, x2t, bt3)
                      if more:
                          nd = prepare_dma(t + 1)
                          dmas.append(nd)
                      ps1 = scores(t, 1, x2t, bt3)
                      if carry is not None:
                          softmax_t(*carry)
                      softmax_t(t, 0, ps0, x2n)
                      if more:
                          jo_red(nd[0], nd[1], 0)
                          prepare_trans(t + 1, nd[0], 0)
                      ps2 = scores(t, 2, x2t, bt3)
                      softmax_t(t, 1, ps1, x2n)
                      if more:
                          jo_red(nd[0], nd[1], 1)
                          prepare_trans(t + 1, nd[0], 1)
                          nx2n = prepare_round(t + 1, nd[0])
                      ps3 = scores(t, 3, x2t, bt3)
                      softmax_t(t, 2, ps2, x2n)
                      if more:
                          jo_red(nd[0], nd[1], 2)
                          prepare_trans(t + 1, nd[0], 2)
                          jo_red(nd[0], nd[1], 3)
                          nx2t = prepare_trans(t + 1, nd[0], 3)
                          nbt3 = b_path(t + 1, nd[1], nd[2])
                          chunks.append((nx2n, nx2t, nbt3))
                      carry = (t, 3, ps3, x2n)
                  softmax_t(*carry)

              # ---------- phase 2: normalize, O = (T/s) @ Wv ----------
              with (
                  tc.tile_pool(name="outp", bufs=2) as outp,
                  tc.tile_pool(name="psP2T", bufs=2, space="PSUM") as psP2T,
                  tc.tile_pool(name="psP2M", bufs=2, space="PSUM") as psP2M,
              ):
                  tt2 = outp.tile([P, CO, NQ], F16, name="tt2", bufs=1)
                  for q in range(QT):
                      ph2_q(q, tt2, outp, psP2T, psP2M)

    nc.compile()
    return nc


_NC_CACHE = None


def _get_nc():
    global _NC_CACHE
    if _NC_CACHE is None:
        _NC_CACHE = build_kernel()
    return _NC_CACHE


def _run(inputs, trace=False):
    """Returns (output [4096,1024] f32, exec_time_ns or None, results obj)."""
    x1 = np.ascontiguousarray(np.asarray(inputs["x_1"], dtype=np.float32))
    x2 = np.ascontiguousarray(np.asarray(inputs["x_2"], dtype=np.float32))
    wq = np.ascontiguousarray(np.asarray(inputs["W_query"], dtype=np.float32))
    wk = np.ascontiguousarray(np.asarray(inputs["W_key"], dtype=np.float32))
    wv = np.ascontiguousarray(np.asarray(inputs["W_value"], dtype=np.float32))

    nc = _get_nc()
    in_maps = [
        {
            "x1s": x1[c * NQ:(c + 1) * NQ],
            "x2": x2,
            "Wq": wq,
            "Wk": wk,
            "Wv": wv,
        }
        for c in range(NCORES)
    ]
    br = run_bass_kernel_spmd(nc, in_maps, list(range(NCORES)), trace=trace)
    out = np.concatenate([br.results[c]["out"] for c in range(NCORES)], axis=0)
    return out.astype(np.float32), br.exec_time_ns, br


def kernel(**inputs) -> np.ndarray:
    out, _, _ = _run(inputs)
    return out

